# revision 1
# baseline (speedup 1.0000x reference)
"""Trainium2 Bass kernel for the RNN-T style Joint network:

    out[b,t,u,v] = sum_k tanh(enc_p[b,t,k] + dec_p[b,u,k] + b1[k]) * W2[v,k] + b2[v]
    enc_p = h_enc @ W1[:, :H].T ; dec_p = h_dec @ W1[:, H:].T

Sharding: data-parallel over B across 8 NeuronCores (B == 8, one batch row per
core). Weights are replicated. No collectives needed.

Per-core pipeline (one NeuronCore):
  GEMM1 (fp32, PE): enc_pT [HID, T] and dec_pT [HID, U] computed directly in
      transposed layout (HID on partitions); b1 folded in via the ScalarE
      per-partition activation bias during PSUM->SBUF evacuation.
  broadcast-add (VectorE): pre[j, t'*64+u] = encbT[j, t] + decT[j, u] in ONE
      tensor_add per [128, 512] block using stride-0 broadcast access
      patterns (verified supported by the DVE).
  tanh (ScalarE): SBUF fp32 -> SBUF bf16, producing hT [HID, TU-chunk] --
      already transposed to be the stationary operand of GEMM2.
  GEMM2 (PE, bf16): out[tu, v] = hT.T @ W2T accumulated over 5 K-tiles in
      fp32 PSUM (1280 N=512 matmuls: the roofline term).
  b2 add (VectorE): PSUM + b2rep -> SBUF fp32 out tile [128, 1024].
  DMA out: contiguous 512KB stores.

The build for chunk c+2 is emitted before GEMM2 of chunk c so the in-order
VectorE queue always runs the next chunk's broadcast-adds ahead of the
current chunk's evacuations, keeping the PE from stalling on hT tiles.
"""

import numpy as np
import ml_dtypes

B, T, U, H = 8, 256, 64, 512
HID, V = 640, 1024
TU = T * U  # 16384
N_CORES = 8
N_CHUNKS = TU // 1024  # 16 chunks of 16 t-values x 64 u-values
KK = HID // 128  # 5 K-tiles

BF16 = ml_dtypes.bfloat16

_CACHE = {}


def _build_bass():
    import concourse.bass as bass
    import concourse.tile as tile
    from concourse import bacc, mybir

    f32 = mybir.dt.float32
    bf16 = mybir.dt.bfloat16
    Tanh = mybir.ActivationFunctionType.Tanh

    nc = bacc.Bacc("TRN2", target_bir_lowering=False, debug=False,
                   num_devices=N_CORES)

    hencT = nc.dram_tensor("hencT", [H, T], bf16, kind="ExternalInput").ap()
    hdecT = nc.dram_tensor("hdecT", [H, U], bf16, kind="ExternalInput").ap()
    w1T = nc.dram_tensor("w1T", [2 * H, HID], bf16, kind="ExternalInput").ap()
    w2T = nc.dram_tensor("w2T", [HID, V], bf16, kind="ExternalInput").ap()
    b1col = nc.dram_tensor("b1col", [HID, 1], f32, kind="ExternalInput").ap()
    b2rep = nc.dram_tensor("b2rep", [128, V], f32, kind="ExternalInput").ap()
    out = nc.dram_tensor("out", [TU, V], f32, kind="ExternalOutput").ap()

    def bcast3(ap2d, mid):
        """[P, N] AP -> [P, mid, N] with a stride-0 middle dim."""
        return bass.AP(tensor=ap2d.tensor, offset=ap2d.offset,
                       ap=[ap2d.ap[0], [0, mid], ap2d.ap[1]])

    def repeat3(ap2d, inner):
        """[P, N] AP -> [P, N, inner] with a stride-0 inner dim."""
        return bass.AP(tensor=ap2d.tensor, offset=ap2d.offset,
                       ap=[ap2d.ap[0], ap2d.ap[1], [0, inner]])

    with tile.TileContext(nc) as tc:
        with (
            tc.tile_pool(name="consts", bufs=1) as consts,
            tc.tile_pool(name="psum", bufs=1, space="PSUM") as psum,
            tc.tile_pool(name="prep", bufs=4) as prep,
            tc.tile_pool(name="hTp", bufs=3) as hTp,
            tc.tile_pool(name="outp", bufs=4) as outp,
        ):
            # ---- load inputs into SBUF ----
            henc_t = []
            for k in range(4):
                t_ = consts.tile([128, T], bf16, tag=f"hencT{k}", name=f"hencT{k}")
                nc.sync.dma_start(out=t_, in_=hencT[k * 128:(k + 1) * 128, :])
                henc_t.append(t_)
            hdec_t = []
            for k in range(4):
                t_ = consts.tile([128, U], bf16, tag=f"hdecT{k}", name=f"hdecT{k}")
                nc.sync.dma_start(out=t_, in_=hdecT[k * 128:(k + 1) * 128, :])
                hdec_t.append(t_)
            b1_t = []
            for kk in range(KK):
                t_ = consts.tile([128, 1], f32, tag=f"b1{kk}", name=f"b1{kk}")
                nc.sync.dma_start(out=t_, in_=b1col[kk * 128:(kk + 1) * 128, :])
                b1_t.append(t_)
            w1_t = []
            for k in range(8):
                t_ = consts.tile([128, HID], bf16, tag=f"w1T{k}", name=f"w1T{k}")
                nc.sync.dma_start(out=t_, in_=w1T[k * 128:(k + 1) * 128, :])
                w1_t.append(t_)
            w2_t = []
            for k in range(KK):
                t_ = consts.tile([128, V], bf16, tag=f"w2T{k}", name=f"w2T{k}")
                nc.gpsimd.dma_start(out=t_, in_=w2T[k * 128:(k + 1) * 128, :])
                w2_t.append(t_)
            b2_t = consts.tile([128, V], f32, tag="b2", name="b2")
            nc.gpsimd.dma_start(out=b2_t, in_=b2rep[:, :])

            # ---- GEMM1 (fp32): enc_pT [HID, T], dec_pT [HID, U] ----
            encbT = []
            decT = []
            for kk in range(KK):
                ps = psum.tile([128, T], f32, tag="g1", bufs=1, name=f"pse{kk}")
                for k in range(4):
                    nc.tensor.matmul(
                        ps,
                        lhsT=w1_t[k][:, kk * 128:(kk + 1) * 128],
                        rhs=henc_t[k],
                        start=(k == 0), stop=(k == 3),
                    )
                e_ = consts.tile([128, T], f32, tag=f"encbT{kk}", name=f"encbT{kk}")
                # encbT = enc_pT + b1 (per-partition bias)
                nc.scalar.add(out=e_, in_=ps, add=b1_t[kk])
                encbT.append(e_)
                psd = psum.tile([128, U], f32, tag="g1d", bufs=1, name=f"psd{kk}")
                for k in range(4):
                    nc.tensor.matmul(
                        psd,
                        lhsT=w1_t[4 + k][:, kk * 128:(kk + 1) * 128],
                        rhs=hdec_t[k],
                        start=(k == 0), stop=(k == 3),
                    )
                d_ = consts.tile([128, U], f32, tag=f"decT{kk}", name=f"decT{kk}")
                nc.scalar.copy(out=d_, in_=psd)
                decT.append(d_)

            # ---- main loop: build is emitted 2 chunks ahead of GEMM2 ----
            hT_by_chunk = {}

            def emit_build(c):
                hts = []
                for kk in range(KK):
                    pre = prep.tile([128, 1024], f32, tag=f"pre{kk}",
                                    name=f"pre{c}_{kk}", bufs=2)
                    pre_ap = pre[:, :]
                    out3 = bass.AP(tensor=pre_ap.tensor, offset=pre_ap.offset,
                                   ap=[pre_ap.ap[0], [64, 16], [1, 64]])
                    nc.vector.tensor_add(
                        out=out3,
                        in0=bcast3(decT[kk][:, :], 16),
                        in1=repeat3(encbT[kk][:, c * 16:(c + 1) * 16], 64),
                    )
                    ht = hTp.tile([128, 1024], bf16, tag=f"hT{kk}",
                                  name=f"hT{c}_{kk}", bufs=3)
                    nc.scalar.activation(out=ht, in_=pre, func=Tanh)
                    hts.append(ht)
                hT_by_chunk[c] = hts

            emit_build(0)
            emit_build(1)
            for c in range(N_CHUNKS):
                if c + 2 < N_CHUNKS:
                    emit_build(c + 2)
                hts = hT_by_chunk.pop(c)
                for mt in range(8):
                    ot = outp.tile([128, V], f32, tag="out", name=f"out{c}_{mt}")
                    ps2 = psum.tile([128, 1024], f32, tag="g2", bufs=3,
                                    name=f"ps2_{c}_{mt}")
                    for vc in range(2):
                        for kk in range(KK):
                            nc.tensor.matmul(
                                ps2[:, vc * 512:(vc + 1) * 512],
                                lhsT=hts[kk][:, mt * 128:(mt + 1) * 128],
                                rhs=w2_t[kk][:, vc * 512:(vc + 1) * 512],
                                start=(kk == 0), stop=(kk == KK - 1),
                            )
                    nc.vector.tensor_add(out=ot, in0=ps2, in1=b2_t)
                    r0 = c * 1024 + mt * 128
                    nc.sync.dma_start(out=out[r0:r0 + 128, :], in_=ot)

    nc.finalize()
    return nc


def _get_nc():
    if "nc" not in _CACHE:
        _CACHE["nc"] = _build_bass()
    return _CACHE["nc"]


def _make_in_maps(h_enc, h_dec, W1, b1, W2, b2):
    h_enc = np.asarray(h_enc, dtype=np.float32)
    h_dec = np.asarray(h_dec, dtype=np.float32)
    W1 = np.asarray(W1, dtype=np.float32)
    b1 = np.asarray(b1, dtype=np.float32)
    W2 = np.asarray(W2, dtype=np.float32)
    b2 = np.asarray(b2, dtype=np.float32)

    w1T = np.ascontiguousarray(W1.T)                    # [2H, HID] f32
    w2T = np.ascontiguousarray(W2.T).astype(BF16)       # [HID, V] bf16
    b1col = np.ascontiguousarray(b1.reshape(HID, 1))
    b2rep = np.ascontiguousarray(np.tile(b2.reshape(1, V), (128, 1)))

    in_maps = []
    for b in range(N_CORES):
        in_maps.append({
            "hencT": np.ascontiguousarray(h_enc[b].T).astype(BF16),  # [H, T]
            "hdecT": np.ascontiguousarray(h_dec[b].T).astype(BF16),  # [H, U]
            "w1T": w1T.astype(BF16),
            "w2T": w2T,
            "b1col": b1col,
            "b2rep": b2rep,
        })
    return in_maps


def _run(in_maps, **kwargs):
    from concourse import bass_utils
    nc = _get_nc()
    return bass_utils.run_bass_kernel_spmd(
        nc, in_maps, core_ids=list(range(N_CORES)), **kwargs)


def kernel(h_enc, h_dec, W1, b1, W2, b2):
    in_maps = _make_in_maps(h_enc, h_dec, W1, b1, W2, b2)
    res = _run(in_maps)
    outs = [r["out"].reshape(T, U, V) for r in res.results]
    return np.stack(outs, axis=0)



# revision 2
# speedup vs baseline: 67099.9434x; 67099.9434x over previous
"""Trainium2 Bass kernel for the RNN-T style Joint network:

    out[b,t,u,v] = sum_k tanh(enc_p[b,t,k] + dec_p[b,u,k] + b1[k]) * W2[v,k] + b2[v]
    enc_p = h_enc @ W1[:, :H].T ; dec_p = h_dec @ W1[:, H:].T

Sharding: data-parallel over B across 8 NeuronCores (B == 8, one batch row per
core). Weights are replicated. No collectives needed.

Per-core pipeline (one NeuronCore):
  GEMM1 (PE, bf16): enc_pT [HID, T] and dec_pT [HID, U] computed directly in
      transposed layout (HID on partitions); b1 folded in via the ScalarE
      per-partition activation bias during PSUM->SBUF evacuation.
  broadcast-add (VectorE): pre[j, t'*64+u] = encbT[j, t] + decT[j, u] in ONE
      tensor_add per [128, 1024] block using stride-0 broadcast access
      patterns.
  tanh (ScalarE): SBUF fp32 -> SBUF bf16, producing hT [HID, tu-chunk] --
      already transposed to be the moving operand of GEMM2.
  GEMM2 (PE, bf16), *flipped*: outT[v, tu] = w2T_blk.T @ hT accumulated over
      5 K-tiles in fp32 PSUM. Per 512-wide tu-block, 8 psum banks hold the 8
      v-blocks of 128; v0-outer / kk-inner order gives each bank ~35 matmuls
      of runway between its last write and its next-block reuse.
  evac (split): psum[v,tu] + b2 (per-partition bias!) -> bf16 SBUF tile.
      Even v-blocks on ScalarE (activation Identity with bias), odd v-blocks
      on VectorE (tensor_add with stride-0 b2 broadcast). Splitting keeps
      both engines well under the PE's ~8.6us per block.
  DMA out (2 HWDGE queues): ScalarE-evacuated tiles go out via the ACT
      queue, VectorE ones via the SP queue. Output is bf16 [V, TU] in DRAM;
      the host transposes and upcasts (fp32 <- bf16 costs ~0.1% rel err,
      well inside tolerance, and halves HBM write traffic).

The build for chunk c+2 (VectorE broadcast-add + ScalarE tanh) is emitted two
1024-wide chunks ahead of GEMM2 so the in-order queues always run ahead of
the PE.
"""

import numpy as np
import ml_dtypes

B, T, U, H = 8, 256, 64, 512
HID, V = 640, 1024
TU = T * U  # 16384
N_CORES = 8
N_CHUNKS = TU // 1024  # 16 chunks of 16 t-values x 64 u-values
N_BLOCKS = TU // 512  # 32 GEMM2 tu-blocks
KK = HID // 128  # 5 K-tiles

BF16 = ml_dtypes.bfloat16

_CACHE = {}


def _build_bass():
    import concourse.bass as bass
    import concourse.tile as tile
    from concourse import bacc, mybir

    f32 = mybir.dt.float32
    bf16 = mybir.dt.bfloat16
    Tanh = mybir.ActivationFunctionType.Tanh

    nc = bacc.Bacc("TRN2", target_bir_lowering=False, debug=False,
                   num_devices=N_CORES)

    hencT = nc.dram_tensor("hencT", [H, T], bf16, kind="ExternalInput").ap()
    hdecT = nc.dram_tensor("hdecT", [H, U], bf16, kind="ExternalInput").ap()
    w1T = nc.dram_tensor("w1T", [2 * H, HID], bf16, kind="ExternalInput").ap()
    w2T = nc.dram_tensor("w2T", [HID, V], bf16, kind="ExternalInput").ap()
    b1col = nc.dram_tensor("b1col", [HID, 1], f32, kind="ExternalInput").ap()
    b2c8 = nc.dram_tensor("b2c8", [128, 8], f32, kind="ExternalInput").ap()
    outT = nc.dram_tensor("outT", [V, TU], bf16, kind="ExternalOutput").ap()

    def bcast3(ap2d, mid):
        """[P, N] AP -> [P, mid, N] with a stride-0 middle dim."""
        return bass.AP(tensor=ap2d.tensor, offset=ap2d.offset,
                       ap=[ap2d.ap[0], [0, mid], ap2d.ap[1]])

    def repeat3(ap2d, inner):
        """[P, N] AP -> [P, N, inner] with a stride-0 inner dim."""
        return bass.AP(tensor=ap2d.tensor, offset=ap2d.offset,
                       ap=[ap2d.ap[0], ap2d.ap[1], [0, inner]])

    def bcast_free(ap2d, n):
        """[P, 1] AP -> [P, n] with a stride-0 free dim."""
        return bass.AP(tensor=ap2d.tensor, offset=ap2d.offset,
                       ap=[ap2d.ap[0], [0, n]])

    with tile.TileContext(nc) as tc:
        with (
            tc.tile_pool(name="consts", bufs=1) as consts,
            tc.tile_pool(name="psum", bufs=1, space="PSUM") as psum,
            tc.tile_pool(name="prep", bufs=4) as prep,
            tc.tile_pool(name="hTp", bufs=3) as hTp,
            tc.tile_pool(name="outp", bufs=6) as outp,
        ):
            # ---- load inputs into SBUF; split across the three DMA paths
            # so the GEMM1-critical tensors land fast ----
            henc_t = []
            for k in range(4):
                t_ = consts.tile([128, T], bf16, tag=f"hencT{k}", name=f"hencT{k}")
                nc.sync.dma_start(out=t_, in_=hencT[k * 128:(k + 1) * 128, :])
                henc_t.append(t_)
            hdec_t = []
            for k in range(4):
                t_ = consts.tile([128, U], bf16, tag=f"hdecT{k}", name=f"hdecT{k}")
                nc.scalar.dma_start(out=t_, in_=hdecT[k * 128:(k + 1) * 128, :])
                hdec_t.append(t_)
            b1_t = []
            for kk in range(KK):
                t_ = consts.tile([128, 1], f32, tag=f"b1{kk}", name=f"b1{kk}")
                nc.scalar.dma_start(out=t_, in_=b1col[kk * 128:(kk + 1) * 128, :])
                b1_t.append(t_)
            w1_t = []
            for k in range(8):
                t_ = consts.tile([128, HID], bf16, tag=f"w1T{k}", name=f"w1T{k}")
                eng = nc.sync if k < 4 else nc.scalar
                eng.dma_start(out=t_, in_=w1T[k * 128:(k + 1) * 128, :])
                w1_t.append(t_)
            w2_t = []
            for k in range(KK):
                t_ = consts.tile([128, V], bf16, tag=f"w2T{k}", name=f"w2T{k}")
                nc.gpsimd.dma_start(out=t_, in_=w2T[k * 128:(k + 1) * 128, :])
                w2_t.append(t_)
            b2_t = consts.tile([128, 8], f32, tag="b2", name="b2")
            nc.gpsimd.dma_start(out=b2_t, in_=b2c8[:, :])

            # ---- GEMM1 (bf16): enc_pT [HID, T], dec_pT [HID, U] ----
            # PSUM tiles share the main loop's 8-bank "mm" tag (sliced).
            encbT = []
            decT = []
            for kk in range(KK):
                ps = psum.tile([128, 512], f32, tag="mm", bufs=8,
                               name=f"pse{kk}")
                for k in range(4):
                    nc.tensor.matmul(
                        ps[:, :T],
                        lhsT=w1_t[k][:, kk * 128:(kk + 1) * 128],
                        rhs=henc_t[k],
                        start=(k == 0), stop=(k == 3),
                    )
                e_ = consts.tile([128, T], f32, tag=f"encbT{kk}", name=f"encbT{kk}")
                # encbT = enc_pT + b1 (per-partition bias)
                nc.scalar.add(out=e_, in_=ps[:, :T], add=b1_t[kk])
                encbT.append(e_)
                psd = psum.tile([128, 512], f32, tag="mm", bufs=8,
                                name=f"psd{kk}")
                for k in range(4):
                    nc.tensor.matmul(
                        psd[:, :U],
                        lhsT=w1_t[4 + k][:, kk * 128:(kk + 1) * 128],
                        rhs=hdec_t[k],
                        start=(k == 0), stop=(k == 3),
                    )
                d_ = consts.tile([128, U], f32, tag=f"decT{kk}", name=f"decT{kk}")
                nc.scalar.copy(out=d_, in_=psd[:, :U])
                decT.append(d_)

            # ---- main loop: build is emitted 2 chunks ahead of GEMM2 ----
            hT_by_chunk = {}

            def emit_build(c):
                hts = []
                for kk in range(KK):
                    pre = prep.tile([128, 1024], f32, tag=f"pre{kk}",
                                    name=f"pre{c}_{kk}", bufs=2)
                    pre_ap = pre[:, :]
                    out3 = bass.AP(tensor=pre_ap.tensor, offset=pre_ap.offset,
                                   ap=[pre_ap.ap[0], [64, 16], [1, 64]])
                    nc.vector.tensor_add(
                        out=out3,
                        in0=bcast3(decT[kk][:, :], 16),
                        in1=repeat3(encbT[kk][:, c * 16:(c + 1) * 16], 64),
                    )
                    ht = hTp.tile([128, 1024], bf16, tag=f"hT{kk}",
                                  name=f"hT{c}_{kk}", bufs=3)
                    nc.scalar.activation(out=ht, in_=pre, func=Tanh)
                    hts.append(ht)
                hT_by_chunk[c] = hts

            emit_build(0)
            emit_build(1)
            for blk in range(N_BLOCKS):
                c, half = divmod(blk, 2)
                if half == 0 and c + 2 < N_CHUNKS:
                    emit_build(c + 2)
                hts = hT_by_chunk[c]
                off = half * 512
                tu0 = blk * 512
                for v0 in range(8):
                    ps = psum.tile([128, 512], f32, tag="mm", bufs=8,
                                   name=f"ps{blk}_{v0}")
                    for kk in range(KK):
                        nc.tensor.matmul(
                            ps,
                            lhsT=w2_t[kk][:, v0 * 128:(v0 + 1) * 128],
                            rhs=hts[kk][:, off:off + 512],
                            start=(kk == 0), stop=(kk == KK - 1),
                        )
                    ot = outp.tile([128, 512], bf16, tag=f"o{v0 % 2}",
                                   bufs=6, name=f"ot{blk}_{v0}")
                    dst = outT[v0 * 128:(v0 + 1) * 128, tu0:tu0 + 512]
                    if v0 % 2 == 0:
                        nc.scalar.add(out=ot, in_=ps, add=b2_t[:, v0:v0 + 1])
                        nc.scalar.dma_start(out=dst, in_=ot)
                    else:
                        nc.vector.tensor_add(
                            out=ot, in0=ps,
                            in1=bcast_free(b2_t[:, v0:v0 + 1], 512))
                        nc.sync.dma_start(out=dst, in_=ot)
                if half == 1:
                    del hT_by_chunk[c]

    nc.finalize()
    return nc


def _get_nc():
    if "nc" not in _CACHE:
        _CACHE["nc"] = _build_bass()
    return _CACHE["nc"]


def _make_in_maps(h_enc, h_dec, W1, b1, W2, b2):
    h_enc = np.asarray(h_enc, dtype=np.float32)
    h_dec = np.asarray(h_dec, dtype=np.float32)
    W1 = np.asarray(W1, dtype=np.float32)
    b1 = np.asarray(b1, dtype=np.float32)
    W2 = np.asarray(W2, dtype=np.float32)
    b2 = np.asarray(b2, dtype=np.float32)

    w1T = np.ascontiguousarray(W1.T).astype(BF16)       # [2H, HID] bf16
    w2T = np.ascontiguousarray(W2.T).astype(BF16)       # [HID, V] bf16
    b1col = np.ascontiguousarray(b1.reshape(HID, 1))
    b2c8 = np.ascontiguousarray(b2.reshape(8, 128).T)   # [128, 8] f32

    in_maps = []
    for b in range(N_CORES):
        in_maps.append({
            "hencT": np.ascontiguousarray(h_enc[b].T).astype(BF16),  # [H, T]
            "hdecT": np.ascontiguousarray(h_dec[b].T).astype(BF16),  # [H, U]
            "w1T": w1T,
            "w2T": w2T,
            "b1col": b1col,
            "b2c8": b2c8,
        })
    return in_maps


def _run(in_maps, **kwargs):
    from concourse import bass_utils
    nc = _get_nc()
    return bass_utils.run_bass_kernel_spmd(
        nc, in_maps, core_ids=list(range(N_CORES)), **kwargs)


def kernel(h_enc, h_dec, W1, b1, W2, b2):
    in_maps = _make_in_maps(h_enc, h_dec, W1, b1, W2, b2)
    res = _run(in_maps)
    outs = [np.asarray(r["outT"]).T.astype(np.float32).reshape(T, U, V)
            for r in res.results]
    return np.stack(outs, axis=0)


# revision 3
# speedup vs baseline: 67848.5834x; 1.0112x over previous
"""Trainium2 Bass kernel for the RNN-T style Joint network:

    out[b,t,u,v] = sum_k tanh(enc_p[b,t,k] + dec_p[b,u,k] + b1[k]) * W2[v,k] + b2[v]
    enc_p = h_enc @ W1[:, :H].T ; dec_p = h_dec @ W1[:, H:].T

Sharding: data-parallel over B across 8 NeuronCores (B == 8, one batch row per
core). Weights are replicated. No collectives needed.

Per-core pipeline (one NeuronCore):
  warmup: ~36 dummy N=128 matmuls on a zeroed tile fill the initial DMA-wait
      window so the PE HAM clock-gate reaches 2.4 GHz before GEMM1.
  inputs: ONE batched DMA per tensor (6 total), spread over the SP and
      gpsimd queues; the compute engines' FIFOs stay free of DMA issue.
  GEMM1 (PE, bf16): enc_pT [HID, T] and dec_pT [HID, U] in transposed layout
      (HID on partitions); b1 folded in via the ScalarE per-partition bias
      during PSUM->SBUF evacuation.
  broadcast-add (VectorE) + tanh (ScalarE): hT [HID, tu-chunk] bf16 via
      stride-0 broadcast access patterns. Chunks 0-1 are built in 512-wide
      halves so the first GEMM2 block starts ~5us earlier; later chunks are
      built 1024-wide, two chunks ahead of consumption.
  GEMM2 (PE, bf16), flipped: outT[v, tu] = w2T_blk.T @ hT accumulated over
      5 K-tiles in fp32 PSUM. Per 512-wide tu-block, 8 psum banks hold the 8
      v-blocks of 128; v0-outer / kk-inner order gives each bank ~35 matmuls
      of runway between its last write and its next-block reuse.
  evac (split): psum[v,tu] + b2 (per-partition bias) -> bf16 into a per-
      parity [128, 2048] staging tile. Even v-blocks on ScalarE (activation
      Identity with bias), odd on VectorE (tensor_add with stride-0 b2).
  DMA out: ONE 512 KB DMA per parity per block (64 total) on the SP queue,
      4 v-blocks per DMA via a 3-D access pattern. Output is bf16 [V, TU];
      the host transposes and upcasts (adds ~0.1% rel err, halves HBM
      writes, and the small DMA/semaphore count shortens the postamble).
"""

import numpy as np
import ml_dtypes

B, T, U, H = 8, 256, 64, 512
HID, V = 640, 1024
TU = T * U  # 16384
N_CORES = 8
N_CHUNKS = TU // 1024  # 16 chunks of 16 t-values x 64 u-values
N_BLOCKS = TU // 512  # 32 GEMM2 tu-blocks
KK = HID // 128  # 5 K-tiles

BF16 = ml_dtypes.bfloat16

_CACHE = {}


def _build_bass():
    import concourse.bass as bass
    import concourse.tile as tile
    from concourse import bacc, mybir

    f32 = mybir.dt.float32
    bf16 = mybir.dt.bfloat16
    Tanh = mybir.ActivationFunctionType.Tanh

    nc = bacc.Bacc("TRN2", target_bir_lowering=False, debug=False,
                   num_devices=N_CORES)

    hencT = nc.dram_tensor("hencT", [H, T], bf16, kind="ExternalInput").ap()
    hdecT = nc.dram_tensor("hdecT", [H, U], bf16, kind="ExternalInput").ap()
    w1T = nc.dram_tensor("w1T", [2 * H, HID], bf16, kind="ExternalInput").ap()
    w2T = nc.dram_tensor("w2T", [HID, V], bf16, kind="ExternalInput").ap()
    b1c5 = nc.dram_tensor("b1c5", [128, KK], f32, kind="ExternalInput").ap()
    b2c8 = nc.dram_tensor("b2c8", [128, 8], f32, kind="ExternalInput").ap()
    outT = nc.dram_tensor("outT", [V, TU], bf16, kind="ExternalOutput").ap()

    def bcast3(ap2d, mid):
        """[P, N] AP -> [P, mid, N] with a stride-0 middle dim."""
        return bass.AP(tensor=ap2d.tensor, offset=ap2d.offset,
                       ap=[ap2d.ap[0], [0, mid], ap2d.ap[1]])

    def repeat3(ap2d, inner):
        """[P, N] AP -> [P, N, inner] with a stride-0 inner dim."""
        return bass.AP(tensor=ap2d.tensor, offset=ap2d.offset,
                       ap=[ap2d.ap[0], ap2d.ap[1], [0, inner]])

    def bcast_free(ap2d, n):
        """[P, 1] AP -> [P, n] with a stride-0 free dim."""
        return bass.AP(tensor=ap2d.tensor, offset=ap2d.offset,
                       ap=[ap2d.ap[0], [0, n]])

    def grouped3(ap2d, gstride, g, inner):
        """[P, >=g*inner] AP -> [P, g, inner] with group stride gstride."""
        return bass.AP(tensor=ap2d.tensor, offset=ap2d.offset,
                       ap=[ap2d.ap[0], [gstride, g], [1, inner]])

    with tile.TileContext(nc) as tc:
        with (
            tc.tile_pool(name="consts", bufs=1) as consts,
            tc.tile_pool(name="psum", bufs=1, space="PSUM") as psum,
            tc.tile_pool(name="prep", bufs=4) as prep,
            tc.tile_pool(name="hTp", bufs=3) as hTp,
            tc.tile_pool(name="outp", bufs=6) as outp,
        ):
            # ---- PE warmup: keep the HAM activity window busy during the
            # initial input-DMA wait so GEMM1 runs at 2.4 GHz ----
            zt = consts.tile([128, 128], bf16, tag="z", name="z")
            nc.scalar.memzero(zt[:, :])
            psw = psum.tile([128, 512], f32, tag="mm", bufs=8, name="warm")
            for _ in range(36):
                nc.tensor.matmul(psw[:, :128], lhsT=zt[:, :], rhs=zt[:, :],
                                 start=True, stop=True)

            # ---- batched input DMAs: one per tensor, on idle queues ----
            hencB = consts.tile([128, 4 * T], bf16, tag="hencB", name="hencB")
            nc.sync.dma_start(
                out=grouped3(hencB[:, :], T, 4, T),
                in_=bass.AP(tensor=hencT.tensor, offset=0,
                            ap=[[T, 128], [128 * T, 4], [1, T]]))
            w1B = consts.tile([128, 8 * HID], bf16, tag="w1B", name="w1B")
            nc.sync.dma_start(
                out=grouped3(w1B[:, :], HID, 8, HID),
                in_=bass.AP(tensor=w1T.tensor, offset=0,
                            ap=[[HID, 128], [128 * HID, 8], [1, HID]]))
            b1_t = consts.tile([128, KK], f32, tag="b1", name="b1")
            nc.sync.dma_start(out=b1_t, in_=b1c5[:, :])

            hdecB = consts.tile([128, 4 * U], bf16, tag="hdecB", name="hdecB")
            nc.gpsimd.dma_start(
                out=grouped3(hdecB[:, :], U, 4, U),
                in_=bass.AP(tensor=hdecT.tensor, offset=0,
                            ap=[[U, 128], [128 * U, 4], [1, U]]))
            w2B = consts.tile([128, KK * V], bf16, tag="w2B", name="w2B")
            nc.gpsimd.dma_start(
                out=grouped3(w2B[:, :], V, KK, V),
                in_=bass.AP(tensor=w2T.tensor, offset=0,
                            ap=[[V, 128], [128 * V, KK], [1, V]]))
            b2_t = consts.tile([128, 8], f32, tag="b2", name="b2")
            nc.gpsimd.dma_start(out=b2_t, in_=b2c8[:, :])

            henc_t = [hencB[:, k * T:(k + 1) * T] for k in range(4)]
            hdec_t = [hdecB[:, k * U:(k + 1) * U] for k in range(4)]
            w1_t = [w1B[:, k * HID:(k + 1) * HID] for k in range(8)]
            w2_t = [w2B[:, k * V:(k + 1) * V] for k in range(KK)]

            # ---- GEMM1 (bf16): enc_pT [HID, T], dec_pT [HID, U] ----
            encbT = []
            decT = []
            for kk in range(KK):
                ps = psum.tile([128, 512], f32, tag="mm", bufs=8,
                               name=f"pse{kk}")
                for k in range(4):
                    nc.tensor.matmul(
                        ps[:, :T],
                        lhsT=w1_t[k][:, kk * 128:(kk + 1) * 128],
                        rhs=henc_t[k],
                        start=(k == 0), stop=(k == 3),
                    )
                e_ = consts.tile([128, T], f32, tag=f"encbT{kk}", name=f"encbT{kk}")
                # encbT = enc_pT + b1 (per-partition bias)
                nc.scalar.add(out=e_, in_=ps[:, :T], add=b1_t[:, kk:kk + 1])
                encbT.append(e_)
                psd = psum.tile([128, 512], f32, tag="mm", bufs=8,
                                name=f"psd{kk}")
                for k in range(4):
                    nc.tensor.matmul(
                        psd[:, :U],
                        lhsT=w1_t[4 + k][:, kk * 128:(kk + 1) * 128],
                        rhs=hdec_t[k],
                        start=(k == 0), stop=(k == 3),
                    )
                d_ = consts.tile([128, U], f32, tag=f"decT{kk}", name=f"decT{kk}")
                nc.scalar.copy(out=d_, in_=psd[:, :U])
                decT.append(d_)

            # ---- hT production ----
            # hblk[blk] = list over kk of (tile, column offset)
            hblk = {}

            def emit_build_half(c, half):
                """512-wide build (8 t-values) -- startup latency path."""
                hts = []
                for kk in range(KK):
                    pre = prep.tile([128, 512], f32, tag=f"preh{kk}",
                                    name=f"preh{c}_{half}_{kk}", bufs=2)
                    pre_ap = pre[:, :]
                    out3 = bass.AP(tensor=pre_ap.tensor, offset=pre_ap.offset,
                                   ap=[pre_ap.ap[0], [64, 8], [1, 64]])
                    t0 = c * 16 + half * 8
                    nc.vector.tensor_add(
                        out=out3,
                        in0=bcast3(decT[kk][:, :], 8),
                        in1=repeat3(encbT[kk][:, t0:t0 + 8], 64),
                    )
                    ht = hTp.tile([128, 512], bf16, tag=f"hTh{kk}",
                                  name=f"hTh{c}_{half}_{kk}", bufs=2)
                    nc.scalar.activation(out=ht, in_=pre, func=Tanh)
                    hts.append((ht, 0))
                hblk[2 * c + half] = hts

            def emit_build(c):
                """1024-wide build (16 t-values) -- steady state."""
                hts = []
                for kk in range(KK):
                    pre = prep.tile([128, 1024], f32, tag=f"pre{kk}",
                                    name=f"pre{c}_{kk}", bufs=2)
                    pre_ap = pre[:, :]
                    out3 = bass.AP(tensor=pre_ap.tensor, offset=pre_ap.offset,
                                   ap=[pre_ap.ap[0], [64, 16], [1, 64]])
                    nc.vector.tensor_add(
                        out=out3,
                        in0=bcast3(decT[kk][:, :], 16),
                        in1=repeat3(encbT[kk][:, c * 16:(c + 1) * 16], 64),
                    )
                    ht = hTp.tile([128, 1024], bf16, tag=f"hT{kk}",
                                  name=f"hT{c}_{kk}", bufs=3)
                    nc.scalar.activation(out=ht, in_=pre, func=Tanh)
                    hts.append(ht)
                hblk[2 * c] = [(ht, 0) for ht in hts]
                hblk[2 * c + 1] = [(ht, 512) for ht in hts]

            for c in range(2):
                for half in range(2):
                    emit_build_half(c, half)

            # ---- main GEMM2 loop ----
            for blk in range(N_BLOCKS):
                c, half = divmod(blk, 2)
                if half == 0 and c + 2 < N_CHUNKS:
                    emit_build(c + 2)
                hts = hblk[blk]
                tu0 = blk * 512
                ot = [None, None]
                for par in range(2):
                    ot[par] = outp.tile([128, 2048], bf16, tag=f"o{par}",
                                        bufs=3, name=f"ot{blk}_{par}")
                for v0 in range(8):
                    ps = psum.tile([128, 512], f32, tag="mm", bufs=8,
                                   name=f"ps{blk}_{v0}")
                    for kk in range(KK):
                        ht, off = hts[kk]
                        nc.tensor.matmul(
                            ps,
                            lhsT=w2_t[kk][:, v0 * 128:(v0 + 1) * 128],
                            rhs=ht[:, off:off + 512],
                            start=(kk == 0), stop=(kk == KK - 1),
                        )
                    par, j = v0 % 2, v0 // 2
                    dst_sl = ot[par][:, j * 512:(j + 1) * 512]
                    if par == 0:
                        nc.scalar.add(out=dst_sl, in_=ps,
                                      add=b2_t[:, v0:v0 + 1])
                    else:
                        nc.vector.tensor_add(
                            out=dst_sl, in0=ps,
                            in1=bcast_free(b2_t[:, v0:v0 + 1], 512))
                # one 512KB DMA per parity: 4 v-blocks via 3-D dst pattern
                for par in range(2):
                    dst = bass.AP(tensor=outT.tensor,
                                  offset=par * 128 * TU + tu0,
                                  ap=[[TU, 128], [256 * TU, 4], [1, 512]])
                    nc.sync.dma_start(out=dst,
                                      in_=grouped3(ot[par][:, :], 512, 4, 512))
                del hblk[blk]

    nc.finalize()
    return nc


def _get_nc():
    if "nc" not in _CACHE:
        _CACHE["nc"] = _build_bass()
    return _CACHE["nc"]


def _make_in_maps(h_enc, h_dec, W1, b1, W2, b2):
    h_enc = np.asarray(h_enc, dtype=np.float32)
    h_dec = np.asarray(h_dec, dtype=np.float32)
    W1 = np.asarray(W1, dtype=np.float32)
    b1 = np.asarray(b1, dtype=np.float32)
    W2 = np.asarray(W2, dtype=np.float32)
    b2 = np.asarray(b2, dtype=np.float32)

    w1T = np.ascontiguousarray(W1.T).astype(BF16)       # [2H, HID] bf16
    w2T = np.ascontiguousarray(W2.T).astype(BF16)       # [HID, V] bf16
    b1c5 = np.ascontiguousarray(b1.reshape(KK, 128).T)  # [128, 5] f32
    b2c8 = np.ascontiguousarray(b2.reshape(8, 128).T)   # [128, 8] f32

    in_maps = []
    for b in range(N_CORES):
        in_maps.append({
            "hencT": np.ascontiguousarray(h_enc[b].T).astype(BF16),  # [H, T]
            "hdecT": np.ascontiguousarray(h_dec[b].T).astype(BF16),  # [H, U]
            "w1T": w1T,
            "w2T": w2T,
            "b1c5": b1c5,
            "b2c8": b2c8,
        })
    return in_maps


def _run(in_maps, **kwargs):
    from concourse import bass_utils
    nc = _get_nc()
    return bass_utils.run_bass_kernel_spmd(
        nc, in_maps, core_ids=list(range(N_CORES)), **kwargs)


def kernel(h_enc, h_dec, W1, b1, W2, b2):
    in_maps = _make_in_maps(h_enc, h_dec, W1, b1, W2, b2)
    res = _run(in_maps)
    outs = [np.asarray(r["outT"]).T.astype(np.float32).reshape(T, U, V)
            for r in res.results]
    return np.stack(outs, axis=0)


# revision 6
# speedup vs baseline: 69470.0610x; 1.0239x over previous
"""Trainium2 Bass kernel for the RNN-T style Joint network:

    out[b,t,u,v] = sum_k tanh(enc_p[b,t,k] + dec_p[b,u,k] + b1[k]) * W2[v,k] + b2[v]
    enc_p = h_enc @ W1[:, :H].T ; dec_p = h_dec @ W1[:, H:].T

Sharding: data-parallel over B across 8 NeuronCores (B == 8, one batch row per
core). Weights are replicated. No collectives needed.

Per-core pipeline (one NeuronCore):
  warmup: ~36 dummy N=128 matmuls on a zeroed tile fill the initial DMA-wait
      window so the PE HAM clock-gate reaches 2.4 GHz before GEMM1.
  inputs: ONE batched DMA per tensor (6 total), spread over the SP and
      gpsimd queues; the compute engines' FIFOs stay free of DMA issue.
  GEMM1 (PE, bf16): enc_pT [HID, T] and dec_pT [HID, U] in transposed layout
      (HID on partitions); b1 folded in via the ScalarE per-partition bias
      during PSUM->SBUF evacuation.
  broadcast-add (VectorE) + tanh (ScalarE): hT [HID, tu-chunk] bf16 via
      stride-0 broadcast access patterns. Chunks 0-1 are built in 512-wide
      halves so the first GEMM2 block starts ~5us earlier; later chunks are
      built 1024-wide, two chunks ahead of consumption.
  GEMM2 (PE, bf16), flipped: outT[v, tu] = w2T_blk.T @ hT accumulated over
      5 K-tiles in fp32 PSUM. Per 512-wide tu-block, 8 psum banks hold the 8
      v-blocks of 128; v0-outer / kk-inner order gives each bank ~35 matmuls
      of runway between its last write and its next-block reuse.
  evac (split): psum[v,tu] + b2 (per-partition bias) -> bf16 into a per-
      parity [128, 2048] staging tile. Even v-blocks on ScalarE (activation
      Identity with bias), odd on VectorE (tensor_add with stride-0 b2).
  DMA out: ONE 512 KB DMA per parity per block (64 total) on the SP queue,
      4 v-blocks per DMA via a 3-D access pattern. Output is bf16 [V, TU];
      the host transposes and upcasts (adds ~0.1% rel err, halves HBM
      writes, and the small DMA/semaphore count shortens the postamble).
"""

import numpy as np
import ml_dtypes

B, T, U, H = 8, 256, 64, 512
HID, V = 640, 1024
TU = T * U  # 16384
N_CORES = 8
N_CHUNKS = TU // 1024  # 16 chunks of 16 t-values x 64 u-values
N_BLOCKS = TU // 512  # 32 GEMM2 tu-blocks
KK = HID // 128  # 5 K-tiles

BF16 = ml_dtypes.bfloat16

_CACHE = {}


def _build_bass():
    import concourse.bass as bass
    import concourse.tile as tile
    from concourse import bacc, mybir

    f32 = mybir.dt.float32
    bf16 = mybir.dt.bfloat16
    Tanh = mybir.ActivationFunctionType.Tanh

    nc = bacc.Bacc("TRN2", target_bir_lowering=False, debug=False,
                   num_devices=N_CORES)

    hencT = nc.dram_tensor("hencT", [H, T], bf16, kind="ExternalInput").ap()
    hdecT = nc.dram_tensor("hdecT", [H, U], bf16, kind="ExternalInput").ap()
    w1T = nc.dram_tensor("w1T", [2 * H, HID], bf16, kind="ExternalInput").ap()
    w2T = nc.dram_tensor("w2T", [HID, V], bf16, kind="ExternalInput").ap()
    b1c5 = nc.dram_tensor("b1c5", [128, KK], f32, kind="ExternalInput").ap()
    b2c8 = nc.dram_tensor("b2c8", [128, 8], f32, kind="ExternalInput").ap()
    outT = nc.dram_tensor("outT", [V, TU], bf16, kind="ExternalOutput").ap()

    def bcast3(ap2d, mid):
        """[P, N] AP -> [P, mid, N] with a stride-0 middle dim."""
        return bass.AP(tensor=ap2d.tensor, offset=ap2d.offset,
                       ap=[ap2d.ap[0], [0, mid], ap2d.ap[1]])

    def repeat3(ap2d, inner):
        """[P, N] AP -> [P, N, inner] with a stride-0 inner dim."""
        return bass.AP(tensor=ap2d.tensor, offset=ap2d.offset,
                       ap=[ap2d.ap[0], ap2d.ap[1], [0, inner]])

    def bcast_free(ap2d, n):
        """[P, 1] AP -> [P, n] with a stride-0 free dim."""
        return bass.AP(tensor=ap2d.tensor, offset=ap2d.offset,
                       ap=[ap2d.ap[0], [0, n]])

    def grouped3(ap2d, gstride, g, inner):
        """[P, >=g*inner] AP -> [P, g, inner] with group stride gstride."""
        return bass.AP(tensor=ap2d.tensor, offset=ap2d.offset,
                       ap=[ap2d.ap[0], [gstride, g], [1, inner]])

    with tile.TileContext(nc) as tc:
        with (
            tc.tile_pool(name="consts", bufs=1) as consts,
            tc.tile_pool(name="psum", bufs=1, space="PSUM") as psum,
            tc.tile_pool(name="prep", bufs=4) as prep,
            tc.tile_pool(name="hTp", bufs=3) as hTp,
            tc.tile_pool(name="outp", bufs=6) as outp,
        ):
            # ---- PE warmup: keep the HAM activity window busy during the
            # initial input-DMA wait so GEMM1 runs at 2.4 GHz. zt is zeroed
            # by gpsimd (ready ~6us, right after the NEFF preamble). ----
            zt = consts.tile([128, 128], bf16, tag="z", name="z")
            nc.gpsimd.memset(zt[:, :], 0)
            psw = psum.tile([128, 512], f32, tag="mm", bufs=8, name="warm")
            for _ in range(36):
                nc.tensor.matmul(psw[:, :128], lhsT=zt[:, :], rhs=zt[:, :],
                                 start=True, stop=True)

            # ---- batched input DMAs, ordered by need-time.
            # sync (HWDGE):   b1, w1-enc-half, henc, w2  (GEMM1-enc critical)
            # scalar (HWDGE): w1-dec-half, hdec          (needed ~2us later)
            # gpsimd:         b2                          (needed at 1st evac)
            b1_t = consts.tile([128, KK], f32, tag="b1", name="b1")
            nc.sync.dma_start(out=b1_t, in_=b1c5[:, :])
            w1aB = consts.tile([128, 4 * HID], bf16, tag="w1aB", name="w1aB")
            nc.sync.dma_start(
                out=grouped3(w1aB[:, :], HID, 4, HID),
                in_=bass.AP(tensor=w1T.tensor, offset=0,
                            ap=[[HID, 128], [128 * HID, 4], [1, HID]]))
            hencB = consts.tile([128, 4 * T], bf16, tag="hencB", name="hencB")
            nc.sync.dma_start(
                out=grouped3(hencB[:, :], T, 4, T),
                in_=bass.AP(tensor=hencT.tensor, offset=0,
                            ap=[[T, 128], [128 * T, 4], [1, T]]))
            w2B = consts.tile([128, KK * V], bf16, tag="w2B", name="w2B")
            nc.sync.dma_start(
                out=grouped3(w2B[:, :], V, KK, V),
                in_=bass.AP(tensor=w2T.tensor, offset=0,
                            ap=[[V, 128], [128 * V, KK], [1, V]]))

            w1bB = consts.tile([128, 4 * HID], bf16, tag="w1bB", name="w1bB")
            nc.scalar.dma_start(
                out=grouped3(w1bB[:, :], HID, 4, HID),
                in_=bass.AP(tensor=w1T.tensor, offset=4 * 128 * HID,
                            ap=[[HID, 128], [128 * HID, 4], [1, HID]]))
            hdecB = consts.tile([128, 4 * U], bf16, tag="hdecB", name="hdecB")
            nc.scalar.dma_start(
                out=grouped3(hdecB[:, :], U, 4, U),
                in_=bass.AP(tensor=hdecT.tensor, offset=0,
                            ap=[[U, 128], [128 * U, 4], [1, U]]))

            b2_t = consts.tile([128, 8], f32, tag="b2", name="b2")
            nc.gpsimd.dma_start(out=b2_t, in_=b2c8[:, :])

            henc_t = [hencB[:, k * T:(k + 1) * T] for k in range(4)]
            hdec_t = [hdecB[:, k * U:(k + 1) * U] for k in range(4)]
            w1_t = ([w1aB[:, k * HID:(k + 1) * HID] for k in range(4)]
                    + [w1bB[:, k * HID:(k + 1) * HID] for k in range(4)])
            w2_t = [w2B[:, k * V:(k + 1) * V] for k in range(KK)]

            # ---- GEMM1 (bf16): enc_pT [HID, T], dec_pT [HID, U] ----
            encbT = []
            decT = []
            for kk in range(KK):
                ps = psum.tile([128, 512], f32, tag="mm", bufs=8,
                               name=f"pse{kk}")
                for k in range(4):
                    nc.tensor.matmul(
                        ps[:, :T],
                        lhsT=w1_t[k][:, kk * 128:(kk + 1) * 128],
                        rhs=henc_t[k],
                        start=(k == 0), stop=(k == 3),
                    )
                e_ = consts.tile([128, T], f32, tag=f"encbT{kk}", name=f"encbT{kk}")
                # encbT = enc_pT + b1 (per-partition bias)
                nc.scalar.add(out=e_, in_=ps[:, :T], add=b1_t[:, kk:kk + 1])
                encbT.append(e_)
                psd = psum.tile([128, 512], f32, tag="mm", bufs=8,
                                name=f"psd{kk}")
                for k in range(4):
                    nc.tensor.matmul(
                        psd[:, :U],
                        lhsT=w1_t[4 + k][:, kk * 128:(kk + 1) * 128],
                        rhs=hdec_t[k],
                        start=(k == 0), stop=(k == 3),
                    )
                d_ = consts.tile([128, U], f32, tag=f"decT{kk}", name=f"decT{kk}")
                nc.scalar.copy(out=d_, in_=psd[:, :U])
                decT.append(d_)

            # ---- hT production ----
            # hblk[blk] = list over kk of (tile, column offset)
            hblk = {}

            def emit_build_half(c, half):
                """512-wide build (8 t-values) -- startup latency path."""
                hts = []
                for kk in range(KK):
                    pre = prep.tile([128, 512], f32, tag=f"preh{kk}",
                                    name=f"preh{c}_{half}_{kk}", bufs=2)
                    pre_ap = pre[:, :]
                    out3 = bass.AP(tensor=pre_ap.tensor, offset=pre_ap.offset,
                                   ap=[pre_ap.ap[0], [64, 8], [1, 64]])
                    t0 = c * 16 + half * 8
                    nc.vector.tensor_add(
                        out=out3,
                        in0=bcast3(decT[kk][:, :], 8),
                        in1=repeat3(encbT[kk][:, t0:t0 + 8], 64),
                    )
                    ht = hTp.tile([128, 512], bf16, tag=f"hTh{kk}",
                                  name=f"hTh{c}_{half}_{kk}", bufs=2)
                    nc.scalar.activation(out=ht, in_=pre, func=Tanh)
                    hts.append((ht, 0))
                hblk[2 * c + half] = hts

            def emit_build(c):
                """1024-wide build (16 t-values) -- steady state."""
                hts = []
                for kk in range(KK):
                    pre = prep.tile([128, 1024], f32, tag=f"pre{kk}",
                                    name=f"pre{c}_{kk}", bufs=2)
                    pre_ap = pre[:, :]
                    out3 = bass.AP(tensor=pre_ap.tensor, offset=pre_ap.offset,
                                   ap=[pre_ap.ap[0], [64, 16], [1, 64]])
                    nc.vector.tensor_add(
                        out=out3,
                        in0=bcast3(decT[kk][:, :], 16),
                        in1=repeat3(encbT[kk][:, c * 16:(c + 1) * 16], 64),
                    )
                    ht = hTp.tile([128, 1024], bf16, tag=f"hT{kk}",
                                  name=f"hT{c}_{kk}", bufs=3)
                    nc.scalar.activation(out=ht, in_=pre, func=Tanh)
                    hts.append(ht)
                hblk[2 * c] = [(ht, 0) for ht in hts]
                hblk[2 * c + 1] = [(ht, 512) for ht in hts]

            emit_build_half(0, 0)
            emit_build_half(0, 1)

            # ---- main GEMM2 loop.  Builds are emitted at the END of each
            # block so PSUM evacuations sit AHEAD of the (slack-rich)
            # broadcast-adds / tanhs in the in-order DVE/ScalarE queues. ----
            for blk in range(N_BLOCKS):
                hts = hblk[blk]
                tu0 = blk * 512
                ot = [None, None]
                for par in range(2):
                    ot[par] = outp.tile([128, 2048], bf16, tag=f"o{par}",
                                        bufs=3, name=f"ot{blk}_{par}")
                for v0 in range(8):
                    ps = psum.tile([128, 512], f32, tag="mm", bufs=8,
                                   name=f"ps{blk}_{v0}")
                    for kk in range(KK):
                        ht, off = hts[kk]
                        nc.tensor.matmul(
                            ps,
                            lhsT=w2_t[kk][:, v0 * 128:(v0 + 1) * 128],
                            rhs=ht[:, off:off + 512],
                            start=(kk == 0), stop=(kk == KK - 1),
                        )
                    par, j = v0 % 2, v0 // 2
                    dst_sl = ot[par][:, j * 512:(j + 1) * 512]
                    if par == 0:
                        nc.scalar.add(out=dst_sl, in_=ps,
                                      add=b2_t[:, v0:v0 + 1])
                    else:
                        nc.vector.tensor_add(
                            out=dst_sl, in0=ps,
                            in1=bcast_free(b2_t[:, v0:v0 + 1], 512))
                # one 512KB DMA per parity (4 v-blocks via 3-D dst pattern),
                # on the queue of the engine that produced the tile
                for par in range(2):
                    dst = bass.AP(tensor=outT.tensor,
                                  offset=par * 128 * TU + tu0,
                                  ap=[[TU, 128], [256 * TU, 4], [1, 512]])
                    eng = nc.scalar if par == 0 else nc.sync
                    eng.dma_start(out=dst,
                                  in_=grouped3(ot[par][:, :], 512, 4, 512))
                del hblk[blk]
                # trailing build for a later block (build(c) lands at the end
                # of blk 2c-3, two blocks before its first consumer blk 2c)
                if blk == 0:
                    emit_build_half(1, 0)
                    emit_build_half(1, 1)
                elif blk % 2 == 1:
                    c_next = (blk - 1) // 2 + 2
                    if c_next < N_CHUNKS:
                        emit_build(c_next)

    nc.finalize()
    return nc


def _get_nc():
    if "nc" not in _CACHE:
        _CACHE["nc"] = _build_bass()
    return _CACHE["nc"]


def _make_in_maps(h_enc, h_dec, W1, b1, W2, b2):
    h_enc = np.asarray(h_enc, dtype=np.float32)
    h_dec = np.asarray(h_dec, dtype=np.float32)
    W1 = np.asarray(W1, dtype=np.float32)
    b1 = np.asarray(b1, dtype=np.float32)
    W2 = np.asarray(W2, dtype=np.float32)
    b2 = np.asarray(b2, dtype=np.float32)

    w1T = np.ascontiguousarray(W1.T).astype(BF16)       # [2H, HID] bf16
    w2T = np.ascontiguousarray(W2.T).astype(BF16)       # [HID, V] bf16
    b1c5 = np.ascontiguousarray(b1.reshape(KK, 128).T)  # [128, 5] f32
    b2c8 = np.ascontiguousarray(b2.reshape(8, 128).T)   # [128, 8] f32

    in_maps = []
    for b in range(N_CORES):
        in_maps.append({
            "hencT": np.ascontiguousarray(h_enc[b].T).astype(BF16),  # [H, T]
            "hdecT": np.ascontiguousarray(h_dec[b].T).astype(BF16),  # [H, U]
            "w1T": w1T,
            "w2T": w2T,
            "b1c5": b1c5,
            "b2c8": b2c8,
        })
    return in_maps


def _run(in_maps, **kwargs):
    from concourse import bass_utils
    nc = _get_nc()
    return bass_utils.run_bass_kernel_spmd(
        nc, in_maps, core_ids=list(range(N_CORES)), **kwargs)


def kernel(h_enc, h_dec, W1, b1, W2, b2):
    in_maps = _make_in_maps(h_enc, h_dec, W1, b1, W2, b2)
    res = _run(in_maps)
    outs = [np.asarray(r["outT"]).T.astype(np.float32).reshape(T, U, V)
            for r in res.results]
    return np.stack(outs, axis=0)


# revision 10
# speedup vs baseline: 70070.9305x; 1.0086x over previous
"""Trainium2 Bass kernel for the RNN-T style Joint network:

    out[b,t,u,v] = sum_k tanh(enc_p[b,t,k] + dec_p[b,u,k] + b1[k]) * W2[v,k] + b2[v]
    enc_p = h_enc @ W1[:, :H].T ; dec_p = h_dec @ W1[:, H:].T

Sharding: data-parallel over B across 8 NeuronCores (B == 8, one batch row per
core). Weights are replicated. No collectives needed.

Per-core pipeline (one NeuronCore):
  warmup: ~36 dummy N=128 matmuls on a zeroed tile fill the initial DMA-wait
      window so the PE HAM clock-gate reaches 2.4 GHz before GEMM1.
  inputs: ONE batched DMA per tensor (6 total), spread over the SP and
      gpsimd queues; the compute engines' FIFOs stay free of DMA issue.
  GEMM1 (PE, bf16): enc_pT [HID, T] and dec_pT [HID, U] in transposed layout
      (HID on partitions); b1 folded in via the ScalarE per-partition bias
      during PSUM->SBUF evacuation.
  broadcast-add (VectorE) + tanh (ScalarE): hT [HID, tu-chunk] bf16 via
      stride-0 broadcast access patterns. Chunks 0-1 are built in 512-wide
      halves so the first GEMM2 block starts ~5us earlier; later chunks are
      built 1024-wide, two chunks ahead of consumption.
  GEMM2 (PE, bf16), flipped: outT[v, tu] = w2T_blk.T @ hT accumulated over
      5 K-tiles in fp32 PSUM. Per 512-wide tu-block, 8 psum banks hold the 8
      v-blocks of 128; v0-outer / kk-inner order gives each bank ~35 matmuls
      of runway between its last write and its next-block reuse.
  evac (split): psum[v,tu] + b2 (per-partition bias) -> bf16 into a per-
      parity [128, 2048] staging tile. Even v-blocks on ScalarE (activation
      Identity with bias), odd on VectorE (tensor_add with stride-0 b2).
  DMA out: ONE 512 KB DMA per parity per block (64 total) on the SP queue,
      4 v-blocks per DMA via a 3-D access pattern. Output is bf16 [V, TU];
      the host transposes and upcasts (adds ~0.1% rel err, halves HBM
      writes, and the small DMA/semaphore count shortens the postamble).
"""

import numpy as np
import ml_dtypes

B, T, U, H = 8, 256, 64, 512
HID, V = 640, 1024
TU = T * U  # 16384
N_CORES = 8
N_CHUNKS = TU // 1024  # 16 chunks of 16 t-values x 64 u-values
N_BLOCKS = TU // 512  # 32 GEMM2 tu-blocks
KK = HID // 128  # 5 K-tiles

BF16 = ml_dtypes.bfloat16

_CACHE = {}


def _build_bass():
    import concourse.bass as bass
    import concourse.tile as tile
    from concourse import bacc, mybir

    f32 = mybir.dt.float32
    bf16 = mybir.dt.bfloat16
    Tanh = mybir.ActivationFunctionType.Tanh

    nc = bacc.Bacc("TRN2", target_bir_lowering=False, debug=False,
                   num_devices=N_CORES)

    # inputs arrive pre-arranged in the exact SBUF layout (partition-major,
    # k-tiles concatenated along the free dim) so each load is ONE fully
    # contiguous 2-D DMA
    hencP = nc.dram_tensor("hencP", [128, 4 * T], bf16, kind="ExternalInput").ap()
    hdecP = nc.dram_tensor("hdecP", [128, 4 * U], bf16, kind="ExternalInput").ap()
    w1aP = nc.dram_tensor("w1aP", [128, 4 * HID], bf16, kind="ExternalInput").ap()
    w1bP = nc.dram_tensor("w1bP", [128, 4 * HID], bf16, kind="ExternalInput").ap()
    w2P = nc.dram_tensor("w2P", [128, KK * V], bf16, kind="ExternalInput").ap()
    b1c5 = nc.dram_tensor("b1c5", [128, KK], f32, kind="ExternalInput").ap()
    b2c8 = nc.dram_tensor("b2c8", [128, 8], f32, kind="ExternalInput").ap()
    outT = nc.dram_tensor("outT", [V, TU], bf16, kind="ExternalOutput").ap()

    def bcast3(ap2d, mid):
        """[P, N] AP -> [P, mid, N] with a stride-0 middle dim."""
        return bass.AP(tensor=ap2d.tensor, offset=ap2d.offset,
                       ap=[ap2d.ap[0], [0, mid], ap2d.ap[1]])

    def repeat3(ap2d, inner):
        """[P, N] AP -> [P, N, inner] with a stride-0 inner dim."""
        return bass.AP(tensor=ap2d.tensor, offset=ap2d.offset,
                       ap=[ap2d.ap[0], ap2d.ap[1], [0, inner]])

    def bcast_free(ap2d, n):
        """[P, 1] AP -> [P, n] with a stride-0 free dim."""
        return bass.AP(tensor=ap2d.tensor, offset=ap2d.offset,
                       ap=[ap2d.ap[0], [0, n]])

    def grouped3(ap2d, gstride, g, inner):
        """[P, >=g*inner] AP -> [P, g, inner] with group stride gstride."""
        return bass.AP(tensor=ap2d.tensor, offset=ap2d.offset,
                       ap=[ap2d.ap[0], [gstride, g], [1, inner]])

    with tile.TileContext(nc) as tc:
        with (
            tc.tile_pool(name="consts", bufs=1) as consts,
            tc.tile_pool(name="psum", bufs=1, space="PSUM") as psum,
            tc.tile_pool(name="prep", bufs=4) as prep,
            tc.tile_pool(name="hTp", bufs=3) as hTp,
            tc.tile_pool(name="outp", bufs=6) as outp,
        ):
            # ---- PE warmup: keep the HAM activity window busy during the
            # initial input-DMA wait so GEMM1 runs at 2.4 GHz. zt is zeroed
            # by gpsimd (ready ~6us, right after the NEFF preamble). ----
            zt = consts.tile([128, 128], bf16, tag="z", name="z")
            nc.gpsimd.memset(zt[:, :], 0)
            psw = psum.tile([128, 512], f32, tag="mm", bufs=8, name="warm")
            for _ in range(36):
                nc.tensor.matmul(psw[:, :128], lhsT=zt[:, :], rhs=zt[:, :],
                                 start=True, stop=True)

            # ---- batched input DMAs, ordered by need-time.
            # sync (HWDGE):   b1, w1-enc-half, henc, w2  (GEMM1-enc critical)
            # scalar (HWDGE): w1-dec-half, hdec          (needed ~2us later)
            # gpsimd:         b2                          (needed at 1st evac)
            b1_t = consts.tile([128, KK], f32, tag="b1", name="b1")
            nc.sync.dma_start(out=b1_t, in_=b1c5[:, :])
            w1aB = consts.tile([128, 4 * HID], bf16, tag="w1aB", name="w1aB")
            nc.sync.dma_start(out=w1aB, in_=w1aP[:, :])
            hencB = consts.tile([128, 4 * T], bf16, tag="hencB", name="hencB")
            nc.sync.dma_start(out=hencB, in_=hencP[:, :])
            w2B = consts.tile([128, KK * V], bf16, tag="w2B", name="w2B")
            nc.sync.dma_start(out=w2B, in_=w2P[:, :])

            w1bB = consts.tile([128, 4 * HID], bf16, tag="w1bB", name="w1bB")
            nc.scalar.dma_start(out=w1bB, in_=w1bP[:, :])
            hdecB = consts.tile([128, 4 * U], bf16, tag="hdecB", name="hdecB")
            nc.scalar.dma_start(out=hdecB, in_=hdecP[:, :])

            b2_t = consts.tile([128, 8], f32, tag="b2", name="b2")
            nc.gpsimd.dma_start(out=b2_t, in_=b2c8[:, :])

            henc_t = [hencB[:, k * T:(k + 1) * T] for k in range(4)]
            hdec_t = [hdecB[:, k * U:(k + 1) * U] for k in range(4)]
            w1_t = ([w1aB[:, k * HID:(k + 1) * HID] for k in range(4)]
                    + [w1bB[:, k * HID:(k + 1) * HID] for k in range(4)])
            w2_t = [w2B[:, k * V:(k + 1) * V] for k in range(KK)]

            # ---- GEMM1 (bf16): enc_pT [HID, T], dec_pT [HID, U] ----
            encbT = []
            decT = []
            for kk in range(KK):
                ps = psum.tile([128, 512], f32, tag="mm", bufs=8,
                               name=f"pse{kk}")
                for k in range(4):
                    nc.tensor.matmul(
                        ps[:, :T],
                        lhsT=w1_t[k][:, kk * 128:(kk + 1) * 128],
                        rhs=henc_t[k],
                        start=(k == 0), stop=(k == 3),
                    )
                e_ = consts.tile([128, T], f32, tag=f"encbT{kk}", name=f"encbT{kk}")
                # encbT = enc_pT + b1 (per-partition bias)
                nc.scalar.add(out=e_, in_=ps[:, :T], add=b1_t[:, kk:kk + 1])
                encbT.append(e_)
                psd = psum.tile([128, 512], f32, tag="mm", bufs=8,
                                name=f"psd{kk}")
                for k in range(4):
                    nc.tensor.matmul(
                        psd[:, :U],
                        lhsT=w1_t[4 + k][:, kk * 128:(kk + 1) * 128],
                        rhs=hdec_t[k],
                        start=(k == 0), stop=(k == 3),
                    )
                d_ = consts.tile([128, U], f32, tag=f"decT{kk}", name=f"decT{kk}")
                nc.scalar.copy(out=d_, in_=psd[:, :U])
                decT.append(d_)

            # ---- hT production ----
            # hblk[blk] = list over kk of (tile, column offset)
            hblk = {}

            def emit_build_half(c, half):
                """512-wide build (8 t-values) -- startup latency path."""
                hts = []
                for kk in range(KK):
                    pre = prep.tile([128, 512], f32, tag=f"preh{kk}",
                                    name=f"preh{c}_{half}_{kk}", bufs=2)
                    pre_ap = pre[:, :]
                    out3 = bass.AP(tensor=pre_ap.tensor, offset=pre_ap.offset,
                                   ap=[pre_ap.ap[0], [64, 8], [1, 64]])
                    t0 = c * 16 + half * 8
                    nc.vector.tensor_add(
                        out=out3,
                        in0=bcast3(decT[kk][:, :], 8),
                        in1=repeat3(encbT[kk][:, t0:t0 + 8], 64),
                    )
                    ht = hTp.tile([128, 512], bf16, tag=f"hTh{kk}",
                                  name=f"hTh{c}_{half}_{kk}", bufs=2)
                    nc.scalar.activation(out=ht, in_=pre, func=Tanh)
                    hts.append((ht, 0))
                hblk[2 * c + half] = hts

            def emit_build(c):
                """1024-wide build (16 t-values) -- steady state."""
                hts = []
                for kk in range(KK):
                    pre = prep.tile([128, 1024], f32, tag=f"pre{kk}",
                                    name=f"pre{c}_{kk}", bufs=2)
                    pre_ap = pre[:, :]
                    out3 = bass.AP(tensor=pre_ap.tensor, offset=pre_ap.offset,
                                   ap=[pre_ap.ap[0], [64, 16], [1, 64]])
                    nc.vector.tensor_add(
                        out=out3,
                        in0=bcast3(decT[kk][:, :], 16),
                        in1=repeat3(encbT[kk][:, c * 16:(c + 1) * 16], 64),
                    )
                    ht = hTp.tile([128, 1024], bf16, tag=f"hT{kk}",
                                  name=f"hT{c}_{kk}", bufs=3)
                    nc.scalar.activation(out=ht, in_=pre, func=Tanh)
                    hts.append(ht)
                hblk[2 * c] = [(ht, 0) for ht in hts]
                hblk[2 * c + 1] = [(ht, 512) for ht in hts]

            emit_build_half(0, 0)
            emit_build_half(0, 1)

            # ---- main GEMM2 loop.  Builds are emitted at the END of each
            # block so PSUM evacuations sit AHEAD of the (slack-rich)
            # broadcast-adds / tanhs in the in-order DVE/ScalarE queues. ----
            for blk in range(N_BLOCKS):
                hts = hblk[blk]
                tu0 = blk * 512
                ot = [None, None]
                for par in range(2):
                    ot[par] = outp.tile([128, 2048], bf16, tag=f"o{par}",
                                        bufs=3, name=f"ot{blk}_{par}")
                for v0 in range(8):
                    ps = psum.tile([128, 512], f32, tag="mm", bufs=8,
                                   name=f"ps{blk}_{v0}")
                    for kk in range(KK):
                        ht, off = hts[kk]
                        nc.tensor.matmul(
                            ps,
                            lhsT=w2_t[kk][:, v0 * 128:(v0 + 1) * 128],
                            rhs=ht[:, off:off + 512],
                            start=(kk == 0), stop=(kk == KK - 1),
                        )
                    par, j = v0 % 2, v0 // 2
                    dst_sl = ot[par][:, j * 512:(j + 1) * 512]
                    if par == 0:
                        nc.scalar.add(out=dst_sl, in_=ps,
                                      add=b2_t[:, v0:v0 + 1])
                    else:
                        nc.vector.tensor_add(
                            out=dst_sl, in0=ps,
                            in1=bcast_free(b2_t[:, v0:v0 + 1], 512))
                # one 512KB DMA per parity (4 v-blocks via 3-D dst pattern),
                # on the queue of the engine that produced the tile. The
                # final block goes out as 256KB halves so the run doesn't
                # end waiting on one long transfer.
                nsplit = 2 if blk == N_BLOCKS - 1 else 1
                for par in range(2):
                    for s in range(nsplit):
                        g = 4 // nsplit
                        dst = bass.AP(tensor=outT.tensor,
                                      offset=(par + 2 * s * g) * 128 * TU + tu0,
                                      ap=[[TU, 128], [256 * TU, g], [1, 512]])
                        eng = nc.scalar if par == 0 else nc.sync
                        eng.dma_start(
                            out=dst,
                            in_=grouped3(ot[par][:, s * g * 512:], 512, g, 512))
                del hblk[blk]
                # trailing build for a later block (build(c) lands at the end
                # of blk 2c-3, two blocks before its first consumer blk 2c)
                if blk == 0:
                    emit_build_half(1, 0)
                    emit_build_half(1, 1)
                elif blk % 2 == 1:
                    c_next = (blk - 1) // 2 + 2
                    if c_next < N_CHUNKS:
                        emit_build(c_next)

    nc.finalize()
    return nc


def _get_nc():
    if "nc" not in _CACHE:
        _CACHE["nc"] = _build_bass()
    return _CACHE["nc"]


def _make_in_maps(h_enc, h_dec, W1, b1, W2, b2):
    h_enc = np.asarray(h_enc, dtype=np.float32)
    h_dec = np.asarray(h_dec, dtype=np.float32)
    W1 = np.asarray(W1, dtype=np.float32)
    b1 = np.asarray(b1, dtype=np.float32)
    W2 = np.asarray(W2, dtype=np.float32)
    b2 = np.asarray(b2, dtype=np.float32)

    def part_major(xT, nk):
        """[nk*128, F] -> [128, nk*F]: partition p holds rows p, p+128, ..."""
        f = xT.shape[1]
        return np.ascontiguousarray(
            xT.reshape(nk, 128, f).transpose(1, 0, 2).reshape(128, nk * f))

    w1T = W1.T.astype(BF16)                             # [2H, HID] bf16
    w1aP = part_major(w1T[:4 * 128], 4)
    w1bP = part_major(w1T[4 * 128:], 4)
    w2P = part_major(W2.T.astype(BF16), KK)             # [128, 5*V]
    b1c5 = np.ascontiguousarray(b1.reshape(KK, 128).T)  # [128, 5] f32
    b2c8 = np.ascontiguousarray(b2.reshape(8, 128).T)   # [128, 8] f32

    in_maps = []
    for b in range(N_CORES):
        in_maps.append({
            "hencP": part_major(h_enc[b].T.astype(BF16), 4),  # [128, 4T]
            "hdecP": part_major(h_dec[b].T.astype(BF16), 4),  # [128, 4U]
            "w1aP": w1aP,
            "w1bP": w1bP,
            "w2P": w2P,
            "b1c5": b1c5,
            "b2c8": b2c8,
        })
    return in_maps


def _run(in_maps, **kwargs):
    from concourse import bass_utils
    nc = _get_nc()
    return bass_utils.run_bass_kernel_spmd(
        nc, in_maps, core_ids=list(range(N_CORES)), **kwargs)


def kernel(h_enc, h_dec, W1, b1, W2, b2):
    in_maps = _make_in_maps(h_enc, h_dec, W1, b1, W2, b2)
    res = _run(in_maps)
    outs = [np.asarray(r["outT"]).T.astype(np.float32).reshape(T, U, V)
            for r in res.results]
    return np.stack(outs, axis=0)
